# revision 1
# baseline (speedup 1.0000x reference)
"""Trainium2 Bass kernel for nn_ExcEmbedding (ragged caption/image cosine sims).

Sharding: caption batch AND image batch both split across 8 cores (32 each).
Per core:
  1. img rows (1152, 1024): leaky-relu (DVE), square (ACT), PE indicator-
     matmul reductions over regions -> img_vectors v (32, 1024).
  2. v -> DRAM -> AllGather -> V (256, 1024); read back and PE-transpose
     into V^T tiles (128 d, 256 i).  (launched before the cap phase so the
     collective flight overlaps it)
  3. cap rows (2048, 1024): same pattern, masked sum + full sum-of-squares
     -> cap_vec cv (32, 1024).
  4. SE gate in transposed layout: h^T = relu(W_sq^T cv^T + b_sq),
     g^T = sigmoid(W_ex^T h^T + b_ex)   (d on partitions, captions free).
  5. sims^T(c, i) via three K=1024 f32r matmuls:
       num = sum_d g*cv*V, vg = sum_d g*V, q2 = sum_d g^2*V^2
     sims = (num*rn + beta*s_c) / (sqrt(q2 + 2*beta*vg + beta^2*D) + eps)
  6. host assembles sims[:, cols_j] = simsT_j.T
"""

import os
import numpy as np

import concourse.bass as bass
import concourse.bacc as bacc
import concourse.mybir as mybir
import concourse.tile as tile
from concourse.bass import _add_dep_helper
from concourse.bass_utils import run_bass_kernel_spmd

F32 = mybir.dt.float32
F32R = mybir.dt.float32r
EPS = 1e-8

NCORES = 8
B = 256
R = 36
T = 64
D = 1024
DSQ = 128
M = B // NCORES          # 32 local captions / images per core
NI = M * R // 128        # 9 img row tiles of (128, D)
NC = M * T // 128        # 16 cap row tiles of (128, D)
KD = D // 128            # 8 d-blocks

USE_F32R = os.environ.get("KERNEL_F32R", "1") == "1"
NOBAR = os.environ.get("KERNEL_NOBAR", "0") == "1"
MM_DT = F32R if USE_F32R else F32


def build_program(beta: float):
    nc = bacc.Bacc("TRN2", target_bir_lowering=False, debug=False,
                   num_devices=NCORES)

    img_rows = nc.dram_tensor("img_rows", [M * R, D], F32, kind="ExternalInput")
    cap_rows = nc.dram_tensor("cap_rows", [M * T, D], F32, kind="ExternalInput")
    # indicator matrices, partition-major: ei_t[p, t*M+c] = E[t*128+p, c]
    ei_t = nc.dram_tensor("ei_t", [128, NI * M], F32, kind="ExternalInput")
    ec2_t = nc.dram_tensor("ec2_t", [128, NC * 2 * M], F32, kind="ExternalInput")
    w_sq = nc.dram_tensor("w_sq", [D, DSQ], F32, kind="ExternalInput")
    w_ex = nc.dram_tensor("w_ex", [DSQ, D], F32, kind="ExternalInput")
    b_sq_t = nc.dram_tensor("b_sq_t", [DSQ, 1], F32, kind="ExternalInput")
    b_ex_t = nc.dram_tensor("b_ex_t", [128, KD], F32, kind="ExternalInput")
    rlens = nc.dram_tensor("rlens", [M, 1], F32, kind="ExternalInput")
    idn128 = nc.dram_tensor("idn128", [128, 128], F32, kind="ExternalInput")
    simst_out = nc.dram_tensor("simst_out", [M, B], F32, kind="ExternalOutput")

    rsem = nc.alloc_semaphore(name="rsem")
    lsem = nc.alloc_semaphore(name="lsem")
    psem = nc.alloc_semaphore(name="psem")

    with tile.TileContext(nc) as tc:
        with (
            tc.tile_pool(name="consts", bufs=1) as consts,
            tc.tile_pool(name="xin", bufs=12) as xin,
            tc.tile_pool(name="ypool", bufs=5) as ypool,
            tc.tile_pool(name="y2pool", bufs=5) as y2pool,
            tc.tile_pool(name="ep", bufs=2) as ep,
            tc.tile_pool(name="smalls", bufs=1) as smalls,
            tc.tile_pool(name="tsb", bufs=1) as tsb,
            tc.tile_pool(name="acc", bufs=2, space="PSUM") as acc,
            tc.tile_pool(name="tps", bufs=2, space="PSUM") as tps,
            tc.tile_pool(name="dram", bufs=1, space="DRAM") as dram,
        ):
            mult = mybir.AluOpType.mult
            amax = mybir.AluOpType.max

            # ---- clear remote sems + kernel barrier (prelude AllGather) ----
            if not NOBAR:
                with tc.tile_critical():
                    nc.gpsimd.sem_clear(rsem)
                    nc.gpsimd.sem_clear(lsem)
                    nc.gpsimd.sem_clear(psem)

            # ---- img indicator (small, needed by the first matmuls) ----
            ei_sb = consts.tile([128, NI, M], MM_DT)
            nc.sync.dma_start(ei_sb[:], ei_t[:].bitcast(MM_DT).rearrange(
                "p (t c) -> p t c", t=NI))
            idn_sb = consts.tile([128, 128], F32)
            nc.sync.dma_start(idn_sb[:], idn128[:])

            # ---- img phase: S1 = sum_r y, S2 = sum_r y^2 per (img, d) ----
            s1 = acc.tile([M, D], F32, tag="acc", name="s1")
            s2 = acc.tile([M, D], F32, tag="acc", name="s2")
            for t in range(NI):
                x = xin.tile([128, D], F32, name="x")
                nc.sync.dma_start(x[:], img_rows[128 * t:128 * (t + 1), :])
                y = ypool.tile([128, D], MM_DT, name="y")
                # leaky_relu(x, 0.1) = max(0.1*x, x)
                nc.vector.scalar_tensor_tensor(y[:], x[:], 0.1, x[:], op0=mult, op1=amax)
                y2 = y2pool.tile([128, D], MM_DT, name="y2")
                nc.scalar.square(y2[:], y[:])
                for h in range(2):
                    cs = slice(512 * h, 512 * (h + 1))
                    nc.tensor.matmul(s1[:, cs], ei_sb[:, t, :], y[:, cs],
                                     start=(t == 0), stop=(t == NI - 1),
                                     skip_group_check=True)
                    nc.tensor.matmul(s2[:, cs], ei_sb[:, t, :], y2[:, cs],
                                     start=(t == 0), stop=(t == NI - 1),
                                     skip_group_check=True)

            # ---- img epilogue: v = (S1/R) / sqrt(S2)  (eps << ulp, dropped) ----
            sqv = ep.tile([M, D], F32, tag="ep", name="sqv")
            nc.scalar.sqrt(sqv[:], s2[:])
            rcpv = ep.tile([M, D], F32, tag="ep", name="rcpv")
            rscr = ep.tile([M, D], F32, tag="rscr", name="rscr")
            nc.vector.reciprocal_approx_accurate(rcpv[:], sqv[:], rscr[:])
            v = smalls.tile([M, D], F32, name="v")
            nc.vector.scalar_tensor_tensor(v[:], rcpv[:], 1.0 / R, s1[:],
                                           op0=mult, op1=mult)

            # ---- share V^T across cores via remote SBUF broadcast ----
            # transpose my v -> my_vt[k] (128 d, 32 i), then each core
            # broadcasts its column block into vt[k] on all 8 cores
            my_vv = tsb.tile([128, KD * 2 * M], MM_DT, name="my_vv")
            for k in range(KD):
                tpm = tps.tile([128, M], F32, tag="t", name="tp")
                nc.tensor.transpose(tpm[:], v[:, 128 * k:128 * (k + 1)],
                                    idn_sb[0:M, 0:M])
                nc.scalar.copy(my_vv[:, 2 * M * k:2 * M * k + M], tpm[:])
                nc.scalar.square(my_vv[:, 2 * M * k + M:2 * M * (k + 1)], tpm[:])
            # ---- cap indicator ----
            ec_sb = consts.tile([128, NC, 2 * M], MM_DT)
            nc.sync.dma_start(ec_sb[:], ec2_t[:].bitcast(MM_DT).rearrange(
                "p (t c) -> p t c", t=NC))

            # ---- cap phase: M1 = masked sum y, S2c = full sum y^2 ----
            m1 = acc.tile([M, D], F32, tag="acc", name="m1")
            s2c = acc.tile([M, D], F32, tag="acc", name="s2c")
            for t in range(NC):
                xc = xin.tile([128, D], F32, name="x")
                nc.sync.dma_start(xc[:], cap_rows[128 * t:128 * (t + 1), :])
                yc = ypool.tile([128, D], MM_DT, name="y")
                nc.vector.scalar_tensor_tensor(yc[:], xc[:], 0.1, xc[:], op0=mult, op1=amax)
                yc2 = y2pool.tile([128, D], MM_DT, name="y2")
                nc.scalar.square(yc2[:], yc[:])
                for h in range(2):
                    cs = slice(512 * h, 512 * (h + 1))
                    nc.tensor.matmul(m1[:, cs], ec_sb[:, t, 0:M], yc[:, cs],
                                     start=(t == 0), stop=(t == NC - 1),
                                     skip_group_check=True)
                    nc.tensor.matmul(s2c[:, cs], ec_sb[:, t, M:2 * M], yc2[:, cs],
                                     start=(t == 0), stop=(t == NC - 1),
                                     skip_group_check=True)

            SEG = KD * 2 * M  # 512 columns per rank segment
            vv = tsb.tile([128, NCORES * SEG], MM_DT, name="vv")
            with tc.tile_critical():
                rank = nc.gpsimd.partition_id()
                nc.gpsimd.remote_dma_broadcast(
                    vv[:, bass.ds(rank * SEG, SEG)], my_vv[:],
                    remote_sem=rsem, local_sem=lsem,
                    rdests=[(0, j) for j in range(NCORES)],
                ).then_inc(psem, 1)
                nc.gpsimd.wait_ge(psem, 1)
                nc.gpsimd.bir_kernel_barrier_wait([list(range(NCORES))])
                nc.gpsimd.trigger_dma(count=1)
                nc.gpsimd.wait_ge(rsem, NCORES * 2)
            # strided views: rank-g columns of block k sit at [g*SEG + 2*M*k (+M)]
            vv4 = vv[:].rearrange("p (g k c) -> p g k c", g=NCORES, k=KD)
            vt = [vv4[:, :, k, 0:M] for k in range(KD)]
            vt2 = [vv4[:, :, k, M:2 * M] for k in range(KD)]

            # ---- late consts (gate weights etc.) ----
            wsq_sb = consts.tile([128, KD, 128], MM_DT)
            nc.sync.dma_start(wsq_sb[:], w_sq.ap().rearrange(
                "(k p) j -> p k j", p=128).bitcast(MM_DT))
            wex_sb = consts.tile([128, D], MM_DT)
            nc.sync.dma_start(wex_sb[:], w_ex[:].bitcast(MM_DT))
            bsq_sb = consts.tile([128, 1], F32)
            nc.sync.dma_start(bsq_sb[:], b_sq_t[:])
            bex_sb = consts.tile([128, KD], F32)
            nc.sync.dma_start(bex_sb[:], b_ex_t[:])
            rlens_sb = consts.tile([M, 1], F32)
            nc.sync.dma_start(rlens_sb[:], rlens[:])

            # ---- cap epilogue: cv = (M1/lens) / sqrt(S2c) ----
            sqc = ep.tile([M, D], F32, tag="ep", name="sqc")
            nc.scalar.sqrt(sqc[:], s2c[:])
            rcpc = ep.tile([M, D], F32, tag="ep", name="rcpc")
            rscr2 = ep.tile([M, D], F32, tag="rscr", name="rscr2")
            nc.vector.reciprocal_approx_accurate(rcpc[:], sqc[:], rscr2[:])
            nc.vector.tensor_scalar_mul(rcpc[:], rcpc[:], rlens_sb[:])
            cv = smalls.tile([M, D], F32, name="cv")
            nc.vector.tensor_mul(cv[:], m1[:], rcpc[:])

            # row stats: cvsum = sum_d cv ; nrm2 = sum_d cv^2
            cvsum = smalls.tile([M, 1], F32, name="cvsum")
            nc.vector.reduce_sum(cvsum[:], cv[:], axis=mybir.AxisListType.X)
            cv2scratch = ep.tile([M, D], F32, tag="ep", name="cv2scratch")
            nrm2 = smalls.tile([M, 1], F32, name="nrm2")
            nc.scalar.square(cv2scratch[:], cv[:])
            nc.vector.reduce_sum(nrm2[:], cv2scratch[:], axis=mybir.AxisListType.X)
            rn0 = smalls.tile([M, 1], F32, name="rn0")
            nc.scalar.sqrt(rn0[:], nrm2[:])
            rn = smalls.tile([M, 1], F32, name="rn")
            nc.vector.reciprocal(rn[:], rn0[:])
            bias_num0 = smalls.tile([M, 1], F32, name="bias_num0")
            nc.vector.tensor_mul(bias_num0[:], cvsum[:], rn[:])
            bias_num = smalls.tile([M, 1], F32, name="bias_num")
            nc.scalar.mul(bias_num[:], bias_num0[:], beta)

            # ---- transpose cv -> cvt[k] (128 d, 32 c) ----
            cvt = []
            for k in range(KD):
                tpc = tps.tile([128, M], F32, tag="t", name="tp")
                nc.tensor.transpose(tpc[:], cv[:, 128 * k:128 * (k + 1)],
                                    idn_sb[0:M, 0:M])
                cvt_k = tsb.tile([128, M], MM_DT, name=f"cvt{k}")
                nc.scalar.copy(cvt_k[:], tpc[:])
                cvt.append(cvt_k)

            # ---- gate in transposed layout ----
            ht_ps = tps.tile([128, M], F32, tag="t", name="ht_ps")
            for k in range(KD):
                nc.tensor.matmul(ht_ps[:], wsq_sb[:, k, :], cvt[k][:],
                                 start=(k == 0), stop=(k == KD - 1),
                                 skip_group_check=True)
            ht = smalls.tile([128, M], MM_DT, name="ht")
            nc.scalar.activation(ht[:], ht_ps[:], mybir.ActivationFunctionType.Relu,
                                 bias=bsq_sb[:], scale=1.0)

            gt, g2t, at = [], [], []
            for k in range(KD):
                gps = tps.tile([128, M], F32, tag="t", name="gps")
                nc.tensor.matmul(gps[:], wex_sb[:, 128 * k:128 * (k + 1)],
                                 ht[:], skip_group_check=True)
                gt_k = tsb.tile([128, M], MM_DT, name=f"gt{k}")
                nc.scalar.activation(gt_k[:], gps[:],
                                     mybir.ActivationFunctionType.Sigmoid,
                                     bias=bex_sb[:, k:k + 1], scale=1.0)
                g2t_k = tsb.tile([128, M], MM_DT, name=f"g2t{k}")
                nc.vector.tensor_mul(g2t_k[:], gt_k[:], gt_k[:])
                at_k = tsb.tile([128, M], MM_DT, name=f"at{k}")
                nc.vector.tensor_mul(at_k[:], gt_k[:], cvt[k][:])
                gt.append(gt_k)
                g2t.append(g2t_k)
                at.append(at_k)

            # ---- final: num/vg/q2 (32 c, 256 i) ----
            num_ps = acc.tile([M, B], F32, tag="acc", name="num_ps")
            vg_ps = acc.tile([M, B], F32, tag="acc", name="vg_ps")
            q2_ps = tps.tile([M, B], F32, tag="t", name="q2_ps")
            for k in range(KD):
                nc.tensor.matmul(num_ps[:], at[k][:], vt[k],
                                 start=(k == 0), stop=(k == KD - 1),
                                 skip_group_check=True)
                nc.tensor.matmul(vg_ps[:], gt[k][:], vt[k],
                                 start=(k == 0), stop=(k == KD - 1),
                                 skip_group_check=True)
                nc.tensor.matmul(q2_ps[:], g2t[k][:], vt2[k],
                                 start=(k == 0), stop=(k == KD - 1),
                                 skip_group_check=True)

            # ---- epilogue: sims^T = (num*rn + beta*s_c) / (sqrt(Q)+eps) ----
            beta2d = smalls.tile([M, 1], F32, name="beta2d")
            nc.vector.memset(beta2d[:], beta * beta * D)
            qt = smalls.tile([M, B], F32, name="qt")
            nc.scalar.activation(qt[:], vg_ps[:], mybir.ActivationFunctionType.Identity,
                                 bias=beta2d[:], scale=2.0 * beta)
            qs = smalls.tile([M, B], F32, name="qs")
            nc.vector.tensor_add(qs[:], qt[:], q2_ps[:])
            sq = smalls.tile([M, B], F32, name="sq")
            nc.scalar.sqrt(sq[:], qs[:])
            rq = smalls.tile([M, B], F32, name="rq")
            rqscr = smalls.tile([M, B], F32, name="rqscr")
            nc.vector.reciprocal_approx_accurate(rq[:], sq[:], rqscr[:])
            nt = smalls.tile([M, B], F32, name="nt")
            nc.scalar.activation(nt[:], num_ps[:], mybir.ActivationFunctionType.Identity,
                                 bias=bias_num[:], scale=rn[:])
            simst = smalls.tile([M, B], F32, name="simst")
            nc.vector.tensor_mul(simst[:], nt[:], rq[:])
            nc.sync.dma_start(simst_out[:], simst[:])

    nc.compile()
    return nc


_PROG_CACHE: dict = {}


def get_program(beta: float, stage: str = "full"):
    key = (beta, USE_F32R)
    if key not in _PROG_CACHE:
        _PROG_CACHE[key] = build_program(beta)
    return _PROG_CACHE[key]


def make_in_maps(img_embed, cap_embed, lens, W_sq, b_sq, W_ex, b_ex):
    img_embed = np.ascontiguousarray(img_embed, dtype=np.float32)
    cap_embed = np.ascontiguousarray(cap_embed, dtype=np.float32)
    lens_i = np.asarray(lens).astype(np.int64)

    w_sq_np = np.ascontiguousarray(W_sq, dtype=np.float32)
    w_ex_np = np.ascontiguousarray(W_ex, dtype=np.float32)
    b_sq_np = np.ascontiguousarray(np.asarray(b_sq, dtype=np.float32).reshape(DSQ, 1))
    b_ex_np = np.ascontiguousarray(
        np.asarray(b_ex, dtype=np.float32).reshape(KD, 128).T)
    idn_np = np.eye(128, dtype=np.float32)

    # image indicator, partition-major: ei_t[p, t*M+c] = 1 if (128t+p)//R == c
    ei_np = np.zeros((M * R, M), dtype=np.float32)
    ei_np[np.arange(M * R), np.arange(M * R) // R] = 1.0
    ei_t_np = np.ascontiguousarray(
        ei_np.reshape(NI, 128, M).transpose(1, 0, 2).reshape(128, NI * M))

    in_maps = []
    for j in range(NCORES):
        sl = slice(M * j, M * (j + 1))
        lens_local = lens_i[sl]
        ec2_np = np.zeros((M * T, 2 * M), dtype=np.float32)
        rows = np.arange(M * T)
        cidx = rows // T
        tidx = rows % T
        ec2_np[rows, M + cidx] = 1.0
        keep = tidx < lens_local[cidx]
        ec2_np[rows[keep], cidx[keep]] = 1.0
        ec2_t_np = np.ascontiguousarray(
            ec2_np.reshape(NC, 128, 2 * M).transpose(1, 0, 2).reshape(128, NC * 2 * M))
        rlens_np = (1.0 / lens_local.astype(np.float32)).reshape(M, 1)

        in_maps.append({
            "img_rows": np.ascontiguousarray(img_embed[sl].reshape(M * R, D)),
            "cap_rows": np.ascontiguousarray(cap_embed[sl].reshape(M * T, D)),
            "ei_t": ei_t_np,
            "ec2_t": ec2_t_np,
            "w_sq": w_sq_np,
            "w_ex": w_ex_np,
            "b_sq_t": b_sq_np,
            "b_ex_t": b_ex_np,
            "rlens": rlens_np,
            "idn128": idn_np,
        })
    return in_maps


LAST_RESULT = None


def kernel(img_embed, cap_embed, lens, W_sq, b_sq, W_ex, b_ex, beta, beta1):
    global LAST_RESULT
    beta_f = float(np.asarray(beta).reshape(-1)[0])
    nc = get_program(beta_f)
    in_maps = make_in_maps(img_embed, cap_embed, lens, W_sq, b_sq, W_ex, b_ex)
    res = run_bass_kernel_spmd(nc, in_maps, core_ids=list(range(NCORES)))
    LAST_RESULT = res
    sims = np.empty((B, B), dtype=np.float32)
    for j in range(NCORES):
        sims[:, M * j:M * (j + 1)] = res.results[j]["simst_out"].T
    return sims

